# revision 4
# baseline (speedup 1.0000x reference)
"""Linear attention (nn_LinearAttention) Trainium2 kernel, 8 NeuronCores.

Sharding: core c handles batch b = c//2 and head-group g = c%2 (8 of 16 heads).
Each core computes its head-group's contribution to out[b] = A_g @ Wo_g; the
host sums the two head-group partials per batch and adds the bias.

Device layout notes:
- Host pre-transposes q/k/v to [DIM, T] so the contraction dim (DIM) lands on
  SBUF partitions for the projection matmuls (PE contracts along partitions).
- Phase 1 computes kf (natural [t, m]) and vh augmented with a ones column
  ([t, 8, 65]); kv_aug[j] = kf_j^T @ [vh_j | 1] is accumulated in PSUM over
  all T in bf16 (column 64 of kv_aug = k_sum).
- Phase 2 computes qf^T ([m, t], fp32r), then per head
  num^T[e,t] = kv_aug_j^T @ qf_j^T with row 64 = denominator; the reciprocal
  of that row is broadcast across partitions with a K=1 ones matmul, the
  normalized A^T feeds Y^T = Wo_g^T @ A^T which is DMA'd out as [DIM, T].
"""

import numpy as np

B = 4
T = 4096
DIM = 1024
HEADS_LOCAL = 8
D = 64
M = HEADS_LOCAL * D  # 512 projection cols per core
P = 128
TB2 = 512            # phase-2 t-block (matmul N)
TB1 = 256            # phase-1 t-block (DMA granularity)
N_CORES = 8

_CACHE = {}


def _build():
    import concourse.tile as tile
    import concourse.mybir as mybir
    from concourse import bacc

    f32 = mybir.dt.float32
    f32r = mybir.dt.float32r
    bf16 = mybir.dt.bfloat16
    AF = mybir.ActivationFunctionType
    OP = mybir.AluOpType

    nc = bacc.Bacc("TRN2", target_bir_lowering=False, debug=False,
                   num_devices=N_CORES)

    xq = nc.dram_tensor("xq", [DIM, T], f32r, kind="ExternalInput")
    xk = nc.dram_tensor("xk", [DIM, T], f32r, kind="ExternalInput")
    xv = nc.dram_tensor("xv", [DIM, T], f32r, kind="ExternalInput")
    wq = nc.dram_tensor("wq", [DIM, M], f32r, kind="ExternalInput")
    wk = nc.dram_tensor("wk", [DIM, M], f32r, kind="ExternalInput")
    wv = nc.dram_tensor("wv", [DIM, M], f32r, kind="ExternalInput")
    wo = nc.dram_tensor("wo", [M, DIM], f32r, kind="ExternalInput")
    yt = nc.dram_tensor("yt", [DIM, T], f32, kind="ExternalOutput")

    NC = DIM // P   # 8 contraction chunks
    NM = M // P     # 4 m-tiles
    NN = DIM // P   # 8 output n-tiles
    NB1 = T // TB1  # 16 phase-1 blocks
    NB2 = T // TB2  # 8 phase-2 blocks
    TS1 = TB1 // P  # 2 t-chunks per phase-1 block

    def feature_map(tmp_pool, out_pool, src_psum, out_dtype, tag):
        # elu(x)+1 == relu(x) + exp(min(x, 0)), exact.
        r_t = tmp_pool.tile([P, 512], f32, tag="r_f")
        nc.scalar.activation(out=r_t[:], in_=src_psum, func=AF.Relu)
        m_t = tmp_pool.tile([P, 512], f32, tag="m_f")
        nc.vector.tensor_tensor(m_t[:], src_psum, r_t[:], OP.subtract)
        e_t = tmp_pool.tile([P, 512], f32, tag="e_f")
        nc.scalar.activation(out=e_t[:], in_=m_t[:], func=AF.Exp)
        o_t = out_pool.tile([P, 512], out_dtype, tag=f"o_{tag}")
        with nc.allow_low_precision(reason="feature map output cast"):
            nc.vector.tensor_tensor(o_t[:], r_t[:], e_t[:], OP.add)
        return o_t

    with tile.TileContext(nc) as tc:
        with (
            tc.tile_pool(name="wpool", bufs=1) as wpool,
            tc.tile_pool(name="xpool", bufs=24) as xpool,
            tc.tile_pool(name="xqpool", bufs=16) as xqpool,
            tc.tile_pool(name="work", bufs=3) as work,
            tc.tile_pool(name="qfpool", bufs=6) as qfpool,
            tc.tile_pool(name="atpool", bufs=6) as atpool,
            tc.tile_pool(name="kvsb", bufs=1) as kvsb_pool,
            tc.tile_pool(name="small", bufs=1) as small,
        ):
            # ---- resident weights ----
            wk_sb = []
            wv_sb = []
            wq_sb = []
            wo_sb = []
            for ci in range(NC):
                t_ = wpool.tile([P, M], f32r, tag=f"wA{ci}")
                nc.sync.dma_start(out=t_[:], in_=wk.ap()[ci * P:(ci + 1) * P, :])
                wk_sb.append(t_)
                t_ = wpool.tile([P, M], f32r, tag=f"wB{ci}")
                nc.sync.dma_start(out=t_[:], in_=wv.ap()[ci * P:(ci + 1) * P, :])
                wv_sb.append(t_)
            for mi in range(NM):
                t_ = wpool.tile([P, DIM], f32r, tag=f"wo{mi}")
                nc.sync.dma_start(out=t_[:], in_=wo.ap()[mi * P:(mi + 1) * P, :])
                wo_sb.append(t_)

            # ones row for the K=1 partition-broadcast matmul
            ones_f32 = small.tile([1, D], f32, tag="ones_f32")
            nc.vector.memset(ones_f32[:], 1.0)
            ones_sb = small.tile([1, D], f32r, tag="ones_f32r")
            nc.scalar.copy(out=ones_sb[:], in_=ones_f32[:])

            # ================= phase 1: k/v -> kv_aug =================
            with (
                tc.tile_pool(name="ps1", bufs=2, space="PSUM") as ps1,
                tc.tile_pool(name="kvps", bufs=1, space="PSUM") as kvps,
            ):
                kv_ps = [kvps.tile([P, 130], f32, tag=f"kv{pr}",
                                   name=f"kv_ps{pr}")
                         for pr in range(4)]
                for tb in range(NB1):
                    xk_b = []
                    xv_b = []
                    for ci in range(NC):
                        t_ = xpool.tile([P, TB1], f32r, tag="x1")
                        nc.sync.dma_start(
                            out=t_[:],
                            in_=xk.ap()[ci * P:(ci + 1) * P,
                                        tb * TB1:(tb + 1) * TB1])
                        xk_b.append(t_)
                        t_ = xpool.tile([P, TB1], f32r, tag="x1")
                        nc.sync.dma_start(
                            out=t_[:],
                            in_=xv.ap()[ci * P:(ci + 1) * P,
                                        tb * TB1:(tb + 1) * TB1])
                        xv_b.append(t_)
                    for tsub in range(TS1):
                        ti = tb * TS1 + tsub
                        tsl = slice(tsub * P, (tsub + 1) * P)
                        psk = ps1.tile([P, M], f32, tag="psk")
                        for ci in range(NC):
                            nc.tensor.matmul(psk[:], xk_b[ci][:, tsl],
                                             wk_sb[ci][:],
                                             start=(ci == 0), stop=(ci == NC - 1))
                        kf_t = feature_map(work, work, psk[:], bf16, "kf")

                        psv = ps1.tile([P, M], f32, tag="psv")
                        for ci in range(NC):
                            nc.tensor.matmul(psv[:], xv_b[ci][:, tsl],
                                             wv_sb[ci][:],
                                             start=(ci == 0), stop=(ci == NC - 1))
                        va_t = work.tile([P, HEADS_LOCAL, D + 1], bf16, tag="va")
                        nc.vector.memset(va_t[:, :, D:D + 1], 1.0)
                        with nc.allow_low_precision(reason="bf16 kv operand"):
                            nc.vector.tensor_copy(
                                out=va_t[:, :, 0:D],
                                in_=psv[:].rearrange("p (h d) -> p h d", d=D))
                        for pr in range(4):
                            nc.tensor.matmul(
                                kv_ps[pr][:],
                                kf_t[:, pr * P:(pr + 1) * P],
                                va_t[:, 2 * pr:2 * pr + 2, :],
                                start=(ti == 0), stop=(ti == NB1 * TS1 - 1))

                kv_sb = []
                for pr in range(4):
                    t_ = kvsb_pool.tile([P, 130], f32r, tag=f"kvsb{pr}")
                    with nc.allow_low_precision(reason="kv state to f32r"):
                        nc.vector.tensor_copy(out=t_[:], in_=kv_ps[pr][:])
                    kv_sb.append(t_)

            # wq reuses the wk slots (phase 1 is done with them)
            for ci in range(NC):
                t_ = wpool.tile([P, M], f32r, tag=f"wA{ci}", name=f"wq_sb{ci}")
                nc.sync.dma_start(out=t_[:], in_=wq.ap()[ci * P:(ci + 1) * P, :])
                wq_sb.append(t_)

            # ================= phase 2: q -> out =================
            with (
                tc.tile_pool(name="psq", bufs=2, space="PSUM") as psq_pool,
                tc.tile_pool(name="psn", bufs=2, space="PSUM") as psn_pool,
                tc.tile_pool(name="psb", bufs=2, space="PSUM") as psb_pool,
                tc.tile_pool(name="psy", bufs=2, space="PSUM") as psy_pool,
            ):
                for tb in range(NB2):
                    xq_b = []
                    for ci in range(NC):
                        t_ = xqpool.tile([P, TB2], f32r, tag="x2")
                        nc.sync.dma_start(
                            out=t_[:],
                            in_=xq.ap()[ci * P:(ci + 1) * P,
                                        tb * TB2:(tb + 1) * TB2])
                        xq_b.append(t_)
                    qf_t = []
                    for mi in range(NM):
                        psq = psq_pool.tile([P, TB2], f32, tag="psq")
                        for ci in range(NC):
                            nc.tensor.matmul(
                                psq[:], wq_sb[ci][:, mi * P:(mi + 1) * P],
                                xq_b[ci][:],
                                start=(ci == 0), stop=(ci == NC - 1))
                        qf_t.append(feature_map(work, qfpool, psq[:], f32r, "qf"))
                    at_t = [atpool.tile([P, TB2], f32r, tag="at",
                                        name=f"at{tb}_{mi_}")
                            for mi_ in range(NM)]
                    for j in range(HEADS_LOCAL):
                        pr, side = j // 2, j % 2
                        if side == 0:
                            kv_j = kv_sb[pr][0:D, 0:D + 1]
                        else:
                            kv_j = kv_sb[pr][D:2 * D, D + 1:2 * (D + 1)]
                        jo = side * D
                        psn = psn_pool.tile([D + 1, TB2], f32, tag="psn")
                        nc.tensor.matmul(psn[:], kv_j, qf_t[pr][jo:jo + D, :],
                                         start=True, stop=True)
                        rc = work.tile([1, TB2], f32r, tag="rc")
                        with nc.allow_low_precision(reason="denominator recip"):
                            nc.vector.reciprocal(out=rc[:], in_=psn[D:D + 1, :])
                        psb = psb_pool.tile([D, TB2], f32, tag="psb")
                        nc.tensor.matmul(psb[:], ones_sb[:], rc[:],
                                         start=True, stop=True)
                        with nc.allow_low_precision(reason="normalized A to f32r"):
                            nc.scalar.copy(out=at_t[pr][jo:jo + D, :],
                                           in_=psn[0:D, :])
                            nc.vector.tensor_tensor(
                                at_t[pr][jo:jo + D, :], at_t[pr][jo:jo + D, :],
                                psb[:], OP.mult)
                    for ni in range(NN):
                        psy = psy_pool.tile([P, TB2], f32, tag="psy")
                        for mi in range(NM):
                            nc.tensor.matmul(
                                psy[:], wo_sb[mi][:, ni * P:(ni + 1) * P],
                                at_t[mi][:],
                                start=(mi == 0), stop=(mi == NM - 1))
                        yt_t = work.tile([P, TB2], f32, tag="yt")
                        nc.scalar.copy(out=yt_t[:], in_=psy[:])
                        nc.sync.dma_start(
                            out=yt.ap()[ni * P:(ni + 1) * P,
                                        tb * TB2:(tb + 1) * TB2],
                            in_=yt_t[:])

    nc.compile()
    return nc


def _get_nc():
    if "nc" not in _CACHE:
        _CACHE["nc"] = _build()
    return _CACHE["nc"]


def kernel(v, k, q, Wq, Wk, Wv, Wo, bo, **extra):
    from concourse.bass_utils import run_bass_kernel_spmd

    v = np.asarray(v, dtype=np.float32)
    k = np.asarray(k, dtype=np.float32)
    q = np.asarray(q, dtype=np.float32)
    Wq = np.asarray(Wq, dtype=np.float32)
    Wk = np.asarray(Wk, dtype=np.float32)
    Wv = np.asarray(Wv, dtype=np.float32)
    Wo = np.asarray(Wo, dtype=np.float32)
    bo = np.asarray(bo, dtype=np.float32)

    nc = _get_nc()

    in_maps = []
    for c in range(N_CORES):
        b, g = c // 2, c % 2
        sl = slice(g * M, (g + 1) * M)
        in_maps.append({
            "xq": np.ascontiguousarray(q[b].T),
            "xk": np.ascontiguousarray(k[b].T),
            "xv": np.ascontiguousarray(v[b].T),
            "wq": np.ascontiguousarray(Wq[:, sl]),
            "wk": np.ascontiguousarray(Wk[:, sl]),
            "wv": np.ascontiguousarray(Wv[:, sl]),
            "wo": np.ascontiguousarray(Wo[sl, :]),
        })

    res = run_bass_kernel_spmd(nc, in_maps, core_ids=list(range(N_CORES)))

    out = np.empty((B, T, DIM), dtype=np.float32)
    for b in range(B):
        acc = res.results[2 * b]["yt"] + res.results[2 * b + 1]["yt"]
        out[b] = acc.T + bo
    return out


# revision 8
# speedup vs baseline: 1.3689x; 1.3689x over previous
"""Linear attention (nn_LinearAttention) Trainium2 kernel, 8 NeuronCores.

Sharding: core c handles batch b = c//2 and head-group g = c%2 (8 of 16 heads).
Each core computes its head-group's contribution to out[b] = A_g @ Wo_g; the
host sums the two head-group partials per batch and adds the bias.

Device layout notes:
- Host pre-transposes q/k/v to [DIM, T] so the contraction dim (DIM) lands on
  SBUF partitions for the projection matmuls (PE contracts along partitions).
- Phase 1 computes kf (natural [t, m]) and vh augmented with a ones column
  ([t, 8, 65]); kv_aug[j] = kf_j^T @ [vh_j | 1] is accumulated in PSUM over
  all T in bf16 (column 64 of kv_aug = k_sum).
- Phase 2 computes qf^T ([m, t], fp32r), then per head
  num^T[e,t] = kv_aug_j^T @ qf_j^T with row 64 = denominator; the reciprocal
  of that row is broadcast across partitions with a K=1 ones matmul, the
  normalized A^T feeds Y^T = Wo_g^T @ A^T which is DMA'd out as [DIM, T].
"""

import numpy as np

B = 4
T = 4096
DIM = 1024
HEADS_LOCAL = 8
D = 64
M = HEADS_LOCAL * D  # 512 projection cols per core
P = 128
TB2 = 512            # phase-2 t-block (matmul N)
TB1 = 256            # phase-1 t-block (DMA granularity)
N_CORES = 8

_CACHE = {}


def _build():
    import concourse.tile as tile
    import concourse.mybir as mybir
    from concourse import bacc

    f32 = mybir.dt.float32
    f32r = mybir.dt.float32r
    bf16 = mybir.dt.bfloat16
    AF = mybir.ActivationFunctionType
    OP = mybir.AluOpType

    nc = bacc.Bacc("TRN2", target_bir_lowering=False, debug=False,
                   num_devices=N_CORES)

    xq = nc.dram_tensor("xq", [DIM, T], bf16, kind="ExternalInput")
    xk = nc.dram_tensor("xk", [DIM, T], bf16, kind="ExternalInput")
    xv = nc.dram_tensor("xv", [DIM, T], bf16, kind="ExternalInput")
    wq = nc.dram_tensor("wq", [DIM, M], bf16, kind="ExternalInput")
    wk = nc.dram_tensor("wk", [DIM, M], bf16, kind="ExternalInput")
    wv = nc.dram_tensor("wv", [DIM, M], bf16, kind="ExternalInput")
    wo = nc.dram_tensor("wo", [M, DIM], bf16, kind="ExternalInput")
    sel = nc.dram_tensor("sel", [8, 4, P], bf16, kind="ExternalInput")
    yt = nc.dram_tensor("yt", [DIM, T], f32, kind="ExternalOutput")

    NC = DIM // P   # 8 contraction chunks
    NM = M // P     # 4 m-tiles
    NN = DIM // P   # 8 output n-tiles
    NB1 = T // TB1  # 16 phase-1 blocks
    NB2 = T // TB2  # 8 phase-2 blocks
    TS1 = TB1 // P  # 2 t-chunks per phase-1 block

    def feature_map(tmp_pool, out_pool, src_psum, out_dtype, tag):
        # elu(x)+1 == relu(x) + exp(min(x, 0)), exact.
        r_t = tmp_pool.tile([P, 512], f32, tag="r_f")
        nc.scalar.activation(out=r_t[:], in_=src_psum, func=AF.Relu)
        m_t = tmp_pool.tile([P, 512], f32, tag="m_f")
        nc.vector.tensor_tensor(m_t[:], src_psum, r_t[:], OP.subtract)
        e_t = tmp_pool.tile([P, 512], f32, tag="e_f")
        nc.scalar.activation(out=e_t[:], in_=m_t[:], func=AF.Exp)
        o_t = out_pool.tile([P, 512], out_dtype, tag=f"o_{tag}")
        with nc.allow_low_precision(reason="feature map output cast"):
            nc.vector.tensor_tensor(o_t[:], r_t[:], e_t[:], OP.add)
        return o_t

    with tile.TileContext(nc) as tc:
        with (
            tc.tile_pool(name="wpool", bufs=1) as wpool,
            tc.tile_pool(name="xpool", bufs=24) as xpool,
            tc.tile_pool(name="xqpool", bufs=16) as xqpool,
            tc.tile_pool(name="work", bufs=3) as work,
            tc.tile_pool(name="qfpool", bufs=6) as qfpool,
            tc.tile_pool(name="atpool", bufs=6) as atpool,
            tc.tile_pool(name="kvsb", bufs=1) as kvsb_pool,
            tc.tile_pool(name="small", bufs=1) as small,
        ):
            # selection matrices for the denominator partition-broadcast
            sel_sb = small.tile([8, 4, P], bf16, tag="sel")
            nc.sync.dma_start(out=sel_sb[:], in_=sel.ap())

            # ---- resident weights ----
            wk_sb = []
            wv_sb = []
            wq_sb = []
            wo_sb = []
            for ci in range(NC):
                t_ = wpool.tile([P, M], bf16, tag=f"wA{ci}")
                nc.sync.dma_start(out=t_[:], in_=wk.ap()[ci * P:(ci + 1) * P, :])
                wk_sb.append(t_)
                t_ = wpool.tile([P, M], bf16, tag=f"wB{ci}")
                nc.sync.dma_start(out=t_[:], in_=wv.ap()[ci * P:(ci + 1) * P, :])
                wv_sb.append(t_)
            for mi in range(NM):
                t_ = wpool.tile([P, DIM], bf16, tag=f"wo{mi}")
                nc.sync.dma_start(out=t_[:], in_=wo.ap()[mi * P:(mi + 1) * P, :])
                wo_sb.append(t_)



            # ================= phase 1: k/v -> kv_aug =================
            with (
                tc.tile_pool(name="ps1", bufs=2, space="PSUM") as ps1,
                tc.tile_pool(name="kvps", bufs=1, space="PSUM") as kvps,
            ):
                kv_ps = [kvps.tile([P, 130], f32, tag=f"kv{pr}",
                                   name=f"kv_ps{pr}")
                         for pr in range(4)]
                for tb in range(NB1):
                    xk_b = []
                    xv_b = []
                    for ci in range(NC):
                        t_ = xpool.tile([P, TB1], bf16, tag="x1")
                        nc.sync.dma_start(
                            out=t_[:],
                            in_=xk.ap()[ci * P:(ci + 1) * P,
                                        tb * TB1:(tb + 1) * TB1])
                        xk_b.append(t_)
                        t_ = xpool.tile([P, TB1], bf16, tag="x1")
                        nc.sync.dma_start(
                            out=t_[:],
                            in_=xv.ap()[ci * P:(ci + 1) * P,
                                        tb * TB1:(tb + 1) * TB1])
                        xv_b.append(t_)
                    for tsub in range(TS1):
                        ti = tb * TS1 + tsub
                        tsl = slice(tsub * P, (tsub + 1) * P)
                        psk = ps1.tile([P, M], f32, tag="psk")
                        for ci in range(NC):
                            nc.tensor.matmul(psk[:], xk_b[ci][:, tsl],
                                             wk_sb[ci][:],
                                             start=(ci == 0), stop=(ci == NC - 1))
                        kf_t = feature_map(work, work, psk[:], bf16, "kf")

                        psv = ps1.tile([P, M], f32, tag="psv")
                        for ci in range(NC):
                            nc.tensor.matmul(psv[:], xv_b[ci][:, tsl],
                                             wv_sb[ci][:],
                                             start=(ci == 0), stop=(ci == NC - 1))
                        va_t = work.tile([P, HEADS_LOCAL, D + 1], bf16, tag="va")
                        nc.vector.memset(va_t[:, :, D:D + 1], 1.0)
                        with nc.allow_low_precision(reason="bf16 kv operand"):
                            nc.vector.tensor_copy(
                                out=va_t[:, :, 0:D],
                                in_=psv[:].rearrange("p (h d) -> p h d", d=D))
                        for pr in range(4):
                            nc.tensor.matmul(
                                kv_ps[pr][:],
                                kf_t[:, pr * P:(pr + 1) * P],
                                va_t[:, 2 * pr:2 * pr + 2, :],
                                start=(ti == 0), stop=(ti == NB1 * TS1 - 1))

                kv_sb = []
                for pr in range(4):
                    t_ = kvsb_pool.tile([P, 130], bf16, tag=f"kvsb{pr}")
                    with nc.allow_low_precision(reason="kv state to bf16"):
                        nc.vector.tensor_copy(out=t_[:], in_=kv_ps[pr][:])
                    kv_sb.append(t_)
                # block-diagonal ksum operand: den[8,t] = ksd[:,mi,:].T @ qf
                ksd = small.tile([P, NM, 8], bf16, tag="ksd")
                nc.vector.memset(ksd[:], 0.0)
                for mi in range(NM):
                    for side in range(2):
                        j = 2 * mi + side
                        src = kv_sb[mi][side * D:(side + 1) * D,
                                        (D if side == 0 else 2 * D + 1):
                                        (D + 1 if side == 0 else 2 * D + 2)]
                        nc.scalar.copy(
                            out=ksd[side * D:(side + 1) * D, mi, j:j + 1],
                            in_=src)

            # wq reuses the wk slots (phase 1 is done with them)
            for ci in range(NC):
                t_ = wpool.tile([P, M], bf16, tag=f"wA{ci}", name=f"wq_sb{ci}")
                nc.sync.dma_start(out=t_[:], in_=wq.ap()[ci * P:(ci + 1) * P, :])
                wq_sb.append(t_)

            # ================= phase 2: q -> out =================
            with (
                tc.tile_pool(name="psq", bufs=2, space="PSUM") as psq_pool,
                tc.tile_pool(name="psn", bufs=2, space="PSUM") as psn_pool,
                tc.tile_pool(name="psb", bufs=2, space="PSUM") as psb_pool,
                tc.tile_pool(name="psy", bufs=2, space="PSUM") as psy_pool,
            ):
                for tb in range(NB2):
                    xq_b = []
                    for ci in range(NC):
                        t_ = xqpool.tile([P, TB2], bf16, tag="x2")
                        nc.sync.dma_start(
                            out=t_[:],
                            in_=xq.ap()[ci * P:(ci + 1) * P,
                                        tb * TB2:(tb + 1) * TB2])
                        xq_b.append(t_)
                    qf_t = []
                    for mi in range(NM):
                        psq = psq_pool.tile([P, TB2], f32, tag="psq")
                        for ci in range(NC):
                            nc.tensor.matmul(
                                psq[:], wq_sb[ci][:, mi * P:(mi + 1) * P],
                                xq_b[ci][:],
                                start=(ci == 0), stop=(ci == NC - 1))
                        qf_t.append(feature_map(work, qfpool, psq[:], bf16, "qf"))
                    at_t = [atpool.tile([P, TB2], bf16, tag="at",
                                        name=f"at{tb}_{mi_}")
                            for mi_ in range(NM)]
                    psden = psb_pool.tile([8, TB2], f32, tag="db")
                    for mi in range(NM):
                        nc.tensor.matmul(psden[:], ksd[:, mi, :], qf_t[mi][:],
                                         start=(mi == 0), stop=(mi == NM - 1))
                    rc = work.tile([8, TB2], bf16, tag="rc")
                    with nc.allow_low_precision(reason="denominator recip"):
                        nc.vector.reciprocal(out=rc[:], in_=psden[:])
                    for j in range(HEADS_LOCAL):
                        pr, side = j // 2, j % 2
                        if side == 0:
                            kv_j = kv_sb[pr][0:D, 0:D]
                        else:
                            kv_j = kv_sb[pr][D:2 * D, D + 1:2 * D + 1]
                        jo = side * D
                        psn = psn_pool.tile([D, TB2], f32, tag="psn")
                        nc.tensor.matmul(psn[:], kv_j, qf_t[pr][jo:jo + D, :],
                                         start=True, stop=True)
                        with nc.allow_low_precision(reason="numerator to bf16"):
                            nc.scalar.copy(out=at_t[pr][jo:jo + D, :],
                                           in_=psn[:])
                    for mi in range(NM):
                        psb = psb_pool.tile([P, TB2], f32, tag="db")
                        nc.tensor.matmul(psb[:], sel_sb[:, mi, :], rc[:],
                                         start=True, stop=True)
                        with nc.allow_low_precision(reason="normalize bf16"):
                            nc.vector.tensor_tensor(
                                at_t[mi][:], at_t[mi][:], psb[:], OP.mult)
                    for ni in range(NN):
                        psy = psy_pool.tile([P, TB2], f32, tag="psy")
                        for mi in range(NM):
                            nc.tensor.matmul(
                                psy[:], wo_sb[mi][:, ni * P:(ni + 1) * P],
                                at_t[mi][:],
                                start=(mi == 0), stop=(mi == NM - 1))
                        yt_t = work.tile([P, TB2], f32, tag="yt")
                        nc.scalar.copy(out=yt_t[:], in_=psy[:])
                        nc.sync.dma_start(
                            out=yt.ap()[ni * P:(ni + 1) * P,
                                        tb * TB2:(tb + 1) * TB2],
                            in_=yt_t[:])

    nc.compile()
    return nc


def _get_nc():
    if "nc" not in _CACHE:
        _CACHE["nc"] = _build()
    return _CACHE["nc"]


def kernel(v, k, q, Wq, Wk, Wv, Wo, bo, **extra):
    from concourse.bass_utils import run_bass_kernel_spmd

    v = np.asarray(v, dtype=np.float32)
    k = np.asarray(k, dtype=np.float32)
    q = np.asarray(q, dtype=np.float32)
    Wq = np.asarray(Wq, dtype=np.float32)
    Wk = np.asarray(Wk, dtype=np.float32)
    Wv = np.asarray(Wv, dtype=np.float32)
    Wo = np.asarray(Wo, dtype=np.float32)
    bo = np.asarray(bo, dtype=np.float32)

    import ml_dtypes

    bf = ml_dtypes.bfloat16
    nc = _get_nc()

    sel = np.zeros((8, 4, P), dtype=bf)
    for mi in range(4):
        sel[2 * mi, mi, 0:D] = 1
        sel[2 * mi + 1, mi, D:2 * D] = 1

    in_maps = []
    for c in range(N_CORES):
        b, g = c // 2, c % 2
        sl = slice(g * M, (g + 1) * M)
        in_maps.append({
            "xq": np.ascontiguousarray(q[b].T.astype(bf)),
            "xk": np.ascontiguousarray(k[b].T.astype(bf)),
            "xv": np.ascontiguousarray(v[b].T.astype(bf)),
            "wq": np.ascontiguousarray(Wq[:, sl].astype(bf)),
            "wk": np.ascontiguousarray(Wk[:, sl].astype(bf)),
            "wv": np.ascontiguousarray(Wv[:, sl].astype(bf)),
            "wo": np.ascontiguousarray(Wo[sl, :].astype(bf)),
            "sel": sel,
        })

    res = run_bass_kernel_spmd(nc, in_maps, core_ids=list(range(N_CORES)))

    out = np.empty((B, T, DIM), dtype=np.float32)
    for b in range(B):
        acc = res.results[2 * b]["yt"] + res.results[2 * b + 1]["yt"]
        out[b] = acc.T + bo
    return out
